# revision 24
# baseline (speedup 1.0000x reference)
"""CPA-loss kernel for 8 TRN2 NeuronCores — v3.

Math: for row b with target t, s[t,t] == 1 collapses the loss to
    loss[b] = -log( e[b,t] / (denom_b + eps) + eps ),
    denom_b = dot(s[t,:], e[b,:]),  e = exp(z)  (max-subtraction skipped).
v3 uses the ln-decomposition  loss[b] ~= ln(denom_b + ~eps) - z[b,t]:
the device only computes  SUM_b ln(denom_b)  (one f32 scalar per core);
the host supplies SUM_b z[b,t] exactly.  Dropping the outer +eps biases
the mean by ~ +eps*E[1/sigma] ~ 1.4e-5 rel; folded into DCORR below.

v3 vs the 42.6us v2 (trace-driven):
 - v2's tail: the [128,1] output DMA emitted 128 4-byte descriptors whose
   completion semaphore posted 7.8us after trigger -> 6.8us dead wait at the
   end-of-kernel barrier.  v3 reduces the per-partition Ln sums to ONE f32
   via a 128x1 ones-matmul and DMAs 4 bytes (1 descriptor).
 - v2's input stream: 2 HWDGE queues, ~900 small descriptors on the sync
   queue (zt/vs/w masks at 128-512B per partition) paced at ~25ns/descriptor
   -> data trickled in until 21us+.  v3 ships only lt (fp8, [100,16384]) in
   4 chunks of 4KB/partition (104 descriptors each, 2 per queue) plus vs and
   the per-block select thresholds PACKED into chunk 0 (one trigger).  zt and
   the [128,128] w masks are gone: masks are built on-device from a
   [1,(m-1)*128] f16 threshold row (ones-matmul partition-broadcast + iota +
   is_ge), exploiting that samples are sorted by class within each block.
 - exp split ACT/DVE as in v2 (two custom-DVE exp2 ops registered at
   import); ACT share re-tuned for the ln-decomposed epilogue.
 - Act tables: dummy Exp on a [1,1] tile right after the ACT-queue triggers
   (table DMA hides under the SBUF fill); dummy Ln after the last ACT exp.
"""

import sys

import ml_dtypes
import numpy as np

for _p in ("/opt/trn_rl_repo",):
    if _p not in sys.path:
        sys.path.append(_p)

import concourse.bass as bass  # noqa: F401
import concourse.tile as tile
from concourse import bacc, mybir
from concourse.bass_utils import run_bass_kernel_spmd
from concourse import dve_ops as _dvo
from concourse.dve_spec import Spec, Src0, Src1, C0, C1, C2, One, lower, _has_src1
from concourse.dve_uop import DveOpSpec

B = 131072
C = 100
CP = 100  # exactly the class rows; eps rides DCORR, pad rows dropped
NCORES = 8
RPC = B // NCORES  # 16384 rows per core
BLK = 128
NBLK = RPC // BLK  # 128 blocks per core
# chunk sizes (blocks): g0,g2 ride the sync queue, g1,g3 the scalar queue.
# Uniform 32-block chunks keep chunk boundaries aligned with the epilogue
# slices (SBLK=64=2 chunks) — staggered sizes measurably hurt (35.6us).
GSIZES = [32, 32, 32, 32]
NCHUNK = len(GSIZES)
CCOLS = GSIZES[0] * BLK  # chunk-0 columns (rides the p0 pack)
SLICES = 2
SBLK = NBLK // SLICES
EPS = 1e-6

F = np.float32
LOG2E = float(np.log2(np.e))
LN2 = float(np.log(2.0))
KMAGIC = 8388735.0  # 2^23 + 127
P23 = 8388608.0     # 2^23
# mean-centered poly 2^f ~ a0*(1 + b1 f + b2 f^2) on [-0.5, 0.5]
B1C = 0.7031777501106262
B2C = 0.23833733797073364
LA = 0.0005543692115323172  # log2(a0), host-applied to DVE columns
# log2-domain logit shift: cancels the fp8-quantization bias on ln(denom)
# (v2-tuned 0.000133; the v3 denominator pipeline is identical).
DCORR = 0.000133
# ln-decomposition drops the two eps terms of the exact formula; their
# combined effect on the mean is +E[ln(1+eps/sigma)] ~ eps*E[1/sigma]
# = +2.1206e-4 absolute (E[1/sigma]~212 for z~N(0,1), long-tailed s).
# Subtracted on the host as a constant.
BIASCORR = 2.1206e-4

# fraction of each chunk's blocks handled by the ACT engine (rest on DVE).
# NOTE: generic tensor ops on GpSimd/Pool measure ~14 ns/elem-col on HW
# (ucode, ~17x the cost-model rate) — do NOT offload exp work there.
ACT_FRAC = 0.72


def _act_blocks(gs: int) -> int:
    return max(1, min(gs - 1, int(round(gs * ACT_FRAC))))


TRACE = False  # test.py flips this to get a profiled run
LAST_RESULTS = None  # stash of the last BassKernelResults (for test.py)

_nc_cache = {}
_ops_cache = []


def _f32(x):
    return np.float32(x)


def _ref1(in0, in1, s0, s1, imm2):
    t = in0.astype(F)
    u = (t + _f32(s0)).astype(F)
    return ((u - _f32(s1)) * _f32(imm2)).astype(F)


def _ref2(in0, in1, s0, s1, imm2):
    t = in0.astype(F)
    u = (t + _f32(imm2)).astype(F)
    n = (u - _f32(imm2)).astype(F)
    f = (t - n).astype(F)
    q = ((_f32(s1) * f).astype(F) + _f32(s0)).astype(F)
    q = (q * f).astype(F)
    q = (q + _f32(1.0)).astype(F)
    return (q * in1.astype(F)).astype(F)


def _register_dve_ops():
    """Register the two exp2 custom-DVE ops (idempotent)."""
    global _ops_cache
    if _ops_cache:
        return _ops_cache
    if "EXP2_BITS_ANT" in _dvo._SUB_OPCODE_FOR_NAME:
        by_name = {o.name: o for o in _dvo.OPS}
        _ops_cache = [by_name["EXP2_BITS_ANT"], by_name["EXP2_FIN_ANT"]]
        return _ops_cache

    def mk(name, body, ref):
        opcode = _dvo._CUSTOM_DVE_ROW_BASE + len(_dvo.OPS)
        spec = Spec(body=body, reference=ref)
        shas = {}
        for ver in ("v3", "v4"):
            ds = DveOpSpec(
                name=name, opcode=opcode, uops=lower(spec, ver=ver),
                rd1_en=_has_src1(spec),
            )
            shas[ver] = ds.sha(ver)
        op = _dvo.DveOp(name, spec, subdim=False, uops_sha=shas)
        _dvo.OPS.append(op)
        _dvo._SUB_OPCODE_FOR_NAME[name] = opcode
        _dvo.CUSTOM_DVE_SPECS[name] = op.spec
        return op

    op1 = mk("EXP2_BITS_ANT", ((Src0 + C0) - C1) * C2, _ref1)
    _u = Src0 + C2
    _n = _u - C2
    _fr = Src0 - _n
    _q = ((C1 * _fr) + C0) * _fr + One
    op2 = mk("EXP2_FIN_ANT", _q * Src1, _ref2)
    _ops_cache = [op1, op2]
    return _ops_cache


def _build_nc(m: int, stride: int):
    op1, op2 = _register_dve_ops()
    nc = bacc.Bacc("TRN2", target_bir_lowering=False, debug=False)
    f32 = mybir.dt.float32
    f16 = mybir.dt.float16
    f8 = mybir.dt.float8e4
    i32 = mybir.dt.int32
    u8 = mybir.dt.uint8

    VSB = m * 256            # vs bytes per partition (f16)
    THB = (m - 1) * 256      # threshold bytes per partition (f16, row 0 only)
    W0 = CCOLS + VSB + THB   # chunk-0 pack bytes per partition

    p0_d = nc.declare_dram_parameter("p0", [CP, W0], u8, isOutput=False)
    c_d = [
        nc.declare_dram_parameter(f"c{i}", [CP, GSIZES[i] * BLK], u8, isOutput=False)
        for i in (1, 2, 3)
    ]
    out_d = nc.declare_dram_parameter("out", [1, 1], f32, isOutput=True)

    with tile.TileContext(nc) as tc:
        with (
            tc.tile_pool(name="const", bufs=1) as cpool,
            tc.tile_pool(name="eta", bufs=2) as etap,
            tc.tile_pool(name="etd", bufs=2) as etdp,
            tc.tile_pool(name="bits", bufs=2) as bitp,
            tc.tile_pool(name="fin", bufs=1) as fin,
            tc.tile_pool(name="res", bufs=1, space="PSUM") as resp,
        ):
            # ---- DMA triggers, earliest first.  sync queue: p0, c2, (out).
            # scalar queue: c1, c3 (the two triggers precede all ACT compute).
            p0_t = cpool.tile([CP, W0], u8, tag="p0", name="p0")
            nc.sync.dma_start(p0_t[:], p0_d[:])
            c_t = [
                cpool.tile([CP, GSIZES[i] * BLK], u8, tag=f"c{i}", name=f"c{i}")
                for i in (1, 2, 3)
            ]
            nc.scalar.dma_start(c_t[0][:], c_d[0][:])
            nc.sync.dma_start(c_t[1][:], c_d[1][:])
            nc.scalar.dma_start(c_t[2][:], c_d[2][:])

            # lt chunk views (fp8): global block g*CBLK + k lives in chunk g
            lt_v = [p0_t[:, 0:CCOLS].bitcast(f8)] + [t[:].bitcast(f8) for t in c_t]
            vs_v = p0_t[:, CCOLS : CCOLS + VSB].bitcast(f16)  # [CP, m*NBLK]
            if m > 1:
                thr_v = p0_t[0:1, CCOLS + VSB : W0].bitcast(f16)  # [1,(m-1)*NBLK]

            # ---- small on-chip constants
            dum = fin.tile([1, 1], f32, tag="dum")
            nc.vector.memset(dum[:], 1.0)
            ones1 = fin.tile([1, BLK], f16, tag="ones1")
            nc.vector.memset(ones1[:], 1.0)
            onesc = fin.tile([BLK, 1], f32, tag="onesc")
            nc.vector.memset(onesc[:], 1.0)
            iota_t = fin.tile([BLK, 1], f32, tag="iota")
            nc.gpsimd.iota(
                iota_t[:], [[0, 1]], base=0, channel_multiplier=1,
                allow_small_or_imprecise_dtypes=True,
            )
            # Table loads hide under the SBUF fill (no data deps).  Load Ln
            # then Exp: if the table RAM holds both, the final Ln needs no
            # reload; if Exp evicts Ln, the late dummy below still prefetches.
            nc.scalar.activation(
                dum[:], dum[:], mybir.ActivationFunctionType.Ln
            )
            nc.scalar.activation(
                dum[:], dum[:], mybir.ActivationFunctionType.Exp
            )

            # ---- partition-broadcast the select thresholds via ones-matmul,
            # then masks gmask_i[p,k] = (p >= thr_i[k])  (samples sorted by
            # class within each block -> staircase select).
            gm = []
            if m > 1:
                thr_ps = resp.tile([BLK, (m - 1) * NBLK], f32, tag="thr")
                nc.tensor.matmul(
                    thr_ps[:], ones1[:], thr_v[:], start=True, stop=True
                )
                for i in range(m - 1):
                    g = fin.tile([BLK, NBLK], u8, tag=f"gm{i}")
                    nc.vector.tensor_tensor(
                        g[:],
                        iota_t[:].to_broadcast([BLK, NBLK]),
                        thr_ps[:, i * NBLK : (i + 1) * NBLK],
                        op=mybir.AluOpType.is_ge,
                    )
                    gm.append(g)

            lnsrc = fin.tile([BLK, NBLK], f32, tag="lnsrc")
            res = [
                resp.tile([BLK, SBLK, stride], f32, tag=f"res{i}", name=f"res{i}")
                for i in range(SLICES)
            ]

            def epilogue(sl):
                cols = slice(sl * SBLK, (sl + 1) * SBLK)
                rsl = res[sl]
                dst = lnsrc[:, cols]
                nc.vector.tensor_copy(dst, rsl[:, :, 0])
                for i in range(1, m):
                    nc.vector.copy_predicated(
                        dst, gm[i - 1][:, cols], rsl[:, :, i]
                    )

            kk = 0
            done = 0
            for g in range(NCHUNK):
                ltg = lt_v[g]
                na = _act_blocks(GSIZES[g])
                nd = GSIZES[g] - na
                ca = na * BLK
                eta = etap.tile([CP, ca], f16, tag="eta")
                nc.scalar.activation(
                    eta[:], ltg[:, :ca], mybir.ActivationFunctionType.Exp,
                    scale=LN2,
                )
                etd = etdp.tile([CP, nd * BLK], f16, tag="etd")
                bits = bitp.tile([CP, nd * BLK], i32, tag="bits")
                nc.vector._custom_dve(
                    op1, out=bits[:], in0=ltg[:, ca:],
                    s0=KMAGIC, s1=P23, imm2=P23,
                )
                nc.vector._custom_dve(
                    op2, out=etd[:], in0=ltg[:, ca:],
                    in1=bits[:].bitcast(mybir.dt.float32),
                    s0=B1C, s1=B2C, imm2=KMAGIC,
                )
                if g == NCHUNK - 1:
                    # prefetch the Ln table behind the last chunk's tail
                    nc.scalar.activation(
                        dum[:], dum[:], mybir.ActivationFunctionType.Ln
                    )
                for k in range(GSIZES[g]):
                    et = eta if k < na else etd
                    koff = k * BLK if k < na else (k - na) * BLK
                    sl, j = kk // SBLK, kk % SBLK
                    nc.tensor.matmul(
                        res[sl][:, j, 0:m],
                        et[:, koff : koff + BLK],
                        vs_v[:, m * kk : m * (kk + 1)],
                        start=True,
                        stop=True,
                    )
                    kk += 1
                while done < SLICES and kk >= (done + 1) * SBLK:
                    epilogue(done)
                    done += 1
            while done < SLICES:
                epilogue(done)
                done += 1

            # ---- SUM_{p,k} ln(denom) -> one f32 scalar
            lnr = fin.tile([BLK, NBLK], f32, tag="lnr")
            lsum = fin.tile([BLK, 1], f32, tag="lsum")
            nc.scalar.activation(
                lnr[:],
                lnsrc[:],
                mybir.ActivationFunctionType.Ln,
                accum_out=lsum[:],
            )
            tot_ps = resp.tile([1, 1], f32, tag="tot")
            nc.tensor.matmul(
                tot_ps[:], onesc[:], lsum[:], start=True, stop=True
            )
            tot_sb = fin.tile([1, 1], f32, tag="totsb")
            nc.vector.tensor_copy(tot_sb[:], tot_ps[:])
            nc.sync.dma_start(out_d[:], tot_sb[:])

    nc.compile()
    return nc


def _pick_stride(m: int) -> int:
    for st in (1, 2, 4, 8, 16):
        if st >= m and 512 % st == 0:
            return st
    raise ValueError(f"too many classes per block: m={m}")


def kernel(logits, s, targets):
    global LAST_RESULTS
    logits = np.asarray(logits, dtype=np.float32)
    s = np.asarray(s, dtype=np.float32)
    t = np.asarray(targets).astype(np.int64).ravel()
    assert logits.shape == (B, C) and s.shape == (C, C) and t.shape == (B,)

    order = np.argsort(t, kind="stable")
    # exact numerator sum on host: SUM_b logits[b, t_b]
    ztsum = float(logits[np.arange(B), t].sum(dtype=np.float64))

    idxs = [order[mm::NCORES] for mm in range(NCORES)]

    m = 1
    block_classes = []
    for idx in idxs:
        tb = t[idx].reshape(NBLK, BLK)
        cs = [np.unique(row) for row in tb]
        m = max(m, max(len(u) for u in cs))
        block_classes.append((tb, cs))
    stride = _pick_stride(m)

    # column ranges (in blocks) handled by the DVE engine per chunk
    bounds = np.cumsum([0] + GSIZES)
    dve_cols = []
    for g in range(NCHUNK):
        na = _act_blocks(GSIZES[g])
        dve_cols.append(((bounds[g] + na) * BLK, bounds[g + 1] * BLK))

    VSB = m * 256
    THB = (m - 1) * 256
    W0 = CCOLS + VSB + THB

    in_maps = []
    for core in range(NCORES):
        idx = idxs[core]
        tb, cs = block_classes[core]
        ltT = np.empty((CP, RPC), dtype=np.float32)
        ltT[:] = (logits[idx].T * LOG2E) + DCORR
        for a, b_ in dve_cols:
            ltT[:, a:b_] += LA
        lt8 = ltT.astype(ml_dtypes.float8_e4m3fn).view(np.uint8)  # [CP, RPC]

        vs = np.zeros((CP, m * NBLK), dtype=np.float16)
        cmat = np.empty((m, NBLK), dtype=np.int64)
        thr = np.full((m - 1, NBLK), BLK, dtype=np.float16)
        for k in range(NBLK):
            u = cs[k]
            cmat[: len(u), k] = u
            cmat[len(u):, k] = u[-1]
            # thresholds: first sample index of candidate i (sorted rows)
            pos = np.searchsorted(tb[k], u)
            for i in range(1, len(u)):
                thr[i - 1, k] = pos[i]
        for i in range(m):
            vs[:, i::m] = s[cmat[i]].T.astype(np.float16)

        p0 = np.zeros((CP, W0), dtype=np.uint8)
        p0[:, :CCOLS] = lt8[:, :CCOLS]
        p0[:, CCOLS : CCOLS + VSB] = vs.view(np.uint8)
        if m > 1:
            p0[0, CCOLS + VSB :] = thr.reshape(1, -1).view(np.uint8)
        im = {"p0": p0}
        for i in (1, 2, 3):
            im[f"c{i}"] = np.ascontiguousarray(
                lt8[:, bounds[i] * BLK : bounds[i + 1] * BLK]
            )
        in_maps.append(im)

    key = (m, stride)
    if key not in _nc_cache:
        _nc_cache[key] = _build_nc(m, stride)
    nc = _nc_cache[key]

    res = run_bass_kernel_spmd(
        nc, in_maps, core_ids=list(range(NCORES)), trace=TRACE
    )
    LAST_RESULTS = res
    lntot = sum(float(r["out"][0, 0]) for r in res.results)
    return np.float32((lntot - ztsum) / B - BIASCORR)


# revision 26
# speedup vs baseline: 1.0261x; 1.0261x over previous
"""CPA-loss kernel for 8 TRN2 NeuronCores — v3.

Math: for row b with target t, s[t,t] == 1 collapses the loss to
    loss[b] = -log( e[b,t] / (denom_b + eps) + eps ),
    denom_b = dot(s[t,:], e[b,:]),  e = exp(z)  (max-subtraction skipped).
v3 uses the ln-decomposition  loss[b] ~= ln(denom_b + ~eps) - z[b,t]:
the device only computes  SUM_b ln(denom_b)  (one f32 scalar per core);
the host supplies SUM_b z[b,t] exactly.  Dropping the outer +eps biases
the mean by ~ +eps*E[1/sigma] ~ 1.4e-5 rel; folded into DCORR below.

v3 vs the 42.6us v2 (trace-driven):
 - v2's tail: the [128,1] output DMA emitted 128 4-byte descriptors whose
   completion semaphore posted 7.8us after trigger -> 6.8us dead wait at the
   end-of-kernel barrier.  v3 reduces the per-partition Ln sums to ONE f32
   via a 128x1 ones-matmul and DMAs 4 bytes (1 descriptor).
 - v2's input stream: 2 HWDGE queues, ~900 small descriptors on the sync
   queue (zt/vs/w masks at 128-512B per partition) paced at ~25ns/descriptor
   -> data trickled in until 21us+.  v3 ships only lt (fp8, [100,16384]) in
   4 chunks of 4KB/partition (104 descriptors each, 2 per queue) plus vs and
   the per-block select thresholds PACKED into chunk 0 (one trigger).  zt and
   the [128,128] w masks are gone: masks are built on-device from a
   [1,(m-1)*128] f16 threshold row (ones-matmul partition-broadcast + iota +
   is_ge), exploiting that samples are sorted by class within each block.
 - exp split ACT/DVE as in v2 (two custom-DVE exp2 ops registered at
   import); ACT share re-tuned for the ln-decomposed epilogue.
 - Act tables: dummy Exp on a [1,1] tile right after the ACT-queue triggers
   (table DMA hides under the SBUF fill); dummy Ln after the last ACT exp.
"""

import sys

import ml_dtypes
import numpy as np

for _p in ("/opt/trn_rl_repo",):
    if _p not in sys.path:
        sys.path.append(_p)

import concourse.bass as bass  # noqa: F401
import concourse.tile as tile
from concourse import bacc, mybir
from concourse.bass_utils import run_bass_kernel_spmd
from concourse import dve_ops as _dvo
from concourse.dve_spec import Spec, Src0, Src1, C0, C1, C2, One, lower, _has_src1
from concourse.dve_uop import DveOpSpec

B = 131072
C = 100
CP = 100  # exactly the class rows; eps rides DCORR, pad rows dropped
NCORES = 8
RPC = B // NCORES  # 16384 rows per core
BLK = 128
NBLK = RPC // BLK  # 128 blocks per core
# chunk sizes (blocks): g0,g2 ride the sync queue, g1,g3 the scalar queue.
# Uniform 32-block chunks keep chunk boundaries aligned with the epilogue
# slices (SBLK=64=2 chunks) — staggered sizes measurably hurt (35.6us).
GSIZES = [32, 32, 32, 32]
NCHUNK = len(GSIZES)
CCOLS = GSIZES[0] * BLK  # chunk-0 columns (rides the p0 pack)
SLICES = 2
SBLK = NBLK // SLICES
EPS = 1e-6

F = np.float32
LOG2E = float(np.log2(np.e))
LN2 = float(np.log(2.0))
KMAGIC = 8388735.0  # 2^23 + 127
P23 = 8388608.0     # 2^23
# mean-centered poly 2^f ~ a0*(1 + b1 f + b2 f^2) on [-0.5, 0.5]
B1C = 0.7031777501106262
B2C = 0.23833733797073364
LA = 0.0005543692115323172  # log2(a0), host-applied to DVE columns
# log2-domain logit shift: cancels the fp8-quantization bias on ln(denom)
# (v2-tuned 0.000133; the v3 denominator pipeline is identical).
DCORR = 0.000133
# ln-decomposition drops the two eps terms of the exact formula; their
# combined effect on the mean is +E[ln(1+eps/sigma)] ~ eps*E[1/sigma]
# = +2.1206e-4 absolute (E[1/sigma]~212 for z~N(0,1), long-tailed s).
# Subtracted on the host as a constant.
BIASCORR = 2.1206e-4

# fraction of each chunk's blocks handled by the ACT engine (rest on DVE).
# NOTE: generic tensor ops on GpSimd/Pool measure ~14 ns/elem-col on HW
# (ucode, ~17x the cost-model rate) — do NOT offload exp work there.
ACT_FRAC = 0.72


def _act_blocks(gs: int) -> int:
    return max(1, min(gs - 1, int(round(gs * ACT_FRAC))))


TRACE = False  # test.py flips this to get a profiled run
LAST_RESULTS = None  # stash of the last BassKernelResults (for test.py)

_nc_cache = {}
_ops_cache = []


def _f32(x):
    return np.float32(x)


def _ref1(in0, in1, s0, s1, imm2):
    t = in0.astype(F)
    u = (t + _f32(s0)).astype(F)
    return ((u - _f32(s1)) * _f32(imm2)).astype(F)


def _ref2(in0, in1, s0, s1, imm2):
    t = in0.astype(F)
    u = (t + _f32(imm2)).astype(F)
    n = (u - _f32(imm2)).astype(F)
    f = (t - n).astype(F)
    q = ((_f32(s1) * f).astype(F) + _f32(s0)).astype(F)
    q = (q * f).astype(F)
    q = (q + _f32(1.0)).astype(F)
    return (q * in1.astype(F)).astype(F)


def _register_dve_ops():
    """Register the two exp2 custom-DVE ops (idempotent)."""
    global _ops_cache
    if _ops_cache:
        return _ops_cache
    if "EXP2_BITS_ANT" in _dvo._SUB_OPCODE_FOR_NAME:
        by_name = {o.name: o for o in _dvo.OPS}
        _ops_cache = [by_name["EXP2_BITS_ANT"], by_name["EXP2_FIN_ANT"]]
        return _ops_cache

    def mk(name, body, ref):
        opcode = _dvo._CUSTOM_DVE_ROW_BASE + len(_dvo.OPS)
        spec = Spec(body=body, reference=ref)
        shas = {}
        for ver in ("v3", "v4"):
            ds = DveOpSpec(
                name=name, opcode=opcode, uops=lower(spec, ver=ver),
                rd1_en=_has_src1(spec),
            )
            shas[ver] = ds.sha(ver)
        op = _dvo.DveOp(name, spec, subdim=False, uops_sha=shas)
        _dvo.OPS.append(op)
        _dvo._SUB_OPCODE_FOR_NAME[name] = opcode
        _dvo.CUSTOM_DVE_SPECS[name] = op.spec
        return op

    op1 = mk("EXP2_BITS_ANT", ((Src0 + C0) - C1) * C2, _ref1)
    _u = Src0 + C2
    _n = _u - C2
    _fr = Src0 - _n
    _q = ((C1 * _fr) + C0) * _fr + One
    op2 = mk("EXP2_FIN_ANT", _q * Src1, _ref2)
    _ops_cache = [op1, op2]
    return _ops_cache


def _build_nc(m: int, stride: int):
    op1, op2 = _register_dve_ops()
    nc = bacc.Bacc("TRN2", target_bir_lowering=False, debug=False)
    f32 = mybir.dt.float32
    f16 = mybir.dt.float16
    f8 = mybir.dt.float8e4
    i32 = mybir.dt.int32
    u8 = mybir.dt.uint8

    VSB = m * 256            # vs bytes per partition (f16)
    THB = (m - 1) * 256      # threshold bytes per partition (f16, row 0 only)
    W0 = CCOLS + VSB + THB   # chunk-0 pack bytes per partition

    p0_d = nc.declare_dram_parameter("p0", [CP, W0], u8, isOutput=False)
    c_d = [
        nc.declare_dram_parameter(f"c{i}", [CP, GSIZES[i] * BLK], u8, isOutput=False)
        for i in (1, 2, 3)
    ]
    out_d = nc.declare_dram_parameter("out", [1, 1], f32, isOutput=True)

    with tile.TileContext(nc) as tc:
        with (
            tc.tile_pool(name="const", bufs=1) as cpool,
            tc.tile_pool(name="eta", bufs=2) as etap,
            tc.tile_pool(name="etd", bufs=2) as etdp,
            tc.tile_pool(name="bits", bufs=2) as bitp,
            tc.tile_pool(name="fin", bufs=1) as fin,
            tc.tile_pool(name="res", bufs=1, space="PSUM") as resp,
        ):
            # ---- DMA triggers, earliest first.  sync queue: p0, c2, (out).
            # scalar queue: c1, c3 (the two triggers precede all ACT compute).
            p0_t = cpool.tile([CP, W0], u8, tag="p0", name="p0")
            nc.sync.dma_start(p0_t[:], p0_d[:])
            c_t = [
                cpool.tile([CP, GSIZES[i] * BLK], u8, tag=f"c{i}", name=f"c{i}")
                for i in (1, 2, 3)
            ]
            nc.scalar.dma_start(c_t[0][:], c_d[0][:])
            nc.sync.dma_start(c_t[1][:], c_d[1][:])
            nc.scalar.dma_start(c_t[2][:], c_d[2][:])

            # lt chunk views (fp8): global block g*CBLK + k lives in chunk g
            lt_v = [p0_t[:, 0:CCOLS].bitcast(f8)] + [t[:].bitcast(f8) for t in c_t]
            vs_v = p0_t[:, CCOLS : CCOLS + VSB].bitcast(f16)  # [CP, m*NBLK]
            if m > 1:
                thr_v = p0_t[0:1, CCOLS + VSB : W0].bitcast(f16)  # [1,(m-1)*NBLK]

            # ---- small on-chip constants
            dum = fin.tile([1, 1], f32, tag="dum")
            nc.vector.memset(dum[:], 1.0)
            ones1 = fin.tile([1, BLK], f16, tag="ones1")
            nc.vector.memset(ones1[:], 1.0)
            onesc = fin.tile([BLK, 1], f32, tag="onesc")
            nc.vector.memset(onesc[:], 1.0)
            iota_t = fin.tile([BLK, 1], f32, tag="iota")
            nc.gpsimd.iota(
                iota_t[:], [[0, 1]], base=0, channel_multiplier=1,
                allow_small_or_imprecise_dtypes=True,
            )
            # Exp table load hides under the SBUF fill (no data deps).
            # (Exp/Ln tables evict each other — measured 4 loads when both
            # are preloaded — so only Exp is preloaded; the final Ln's
            # table load is a no-wait aux op that runs in ACT's idle tail.)
            nc.scalar.activation(
                dum[:], dum[:], mybir.ActivationFunctionType.Exp
            )

            # ---- partition-broadcast the select thresholds via ones-matmul,
            # then masks gmask_i[p,k] = (p >= thr_i[k])  (samples sorted by
            # class within each block -> staircase select).
            gm = []
            if m > 1:
                thr_ps = resp.tile([BLK, (m - 1) * NBLK], f32, tag="thr")
                nc.tensor.matmul(
                    thr_ps[:], ones1[:], thr_v[:], start=True, stop=True
                )
                for i in range(m - 1):
                    g = fin.tile([BLK, NBLK], u8, tag=f"gm{i}")
                    nc.vector.tensor_tensor(
                        g[:],
                        iota_t[:].to_broadcast([BLK, NBLK]),
                        thr_ps[:, i * NBLK : (i + 1) * NBLK],
                        op=mybir.AluOpType.is_ge,
                    )
                    gm.append(g)

            lnsrc = fin.tile([BLK, NBLK], f32, tag="lnsrc")
            res = [
                resp.tile([BLK, SBLK, stride], f32, tag=f"res{i}", name=f"res{i}")
                for i in range(SLICES)
            ]

            def epilogue(sl):
                cols = slice(sl * SBLK, (sl + 1) * SBLK)
                rsl = res[sl]
                dst = lnsrc[:, cols]
                nc.vector.tensor_copy(dst, rsl[:, :, 0])
                for i in range(1, m):
                    nc.vector.copy_predicated(
                        dst, gm[i - 1][:, cols], rsl[:, :, i]
                    )

            kk = 0
            done = 0
            for g in range(NCHUNK):
                ltg = lt_v[g]
                na = _act_blocks(GSIZES[g])
                nd = GSIZES[g] - na
                ca = na * BLK
                eta = etap.tile([CP, ca], f16, tag="eta")
                nc.scalar.activation(
                    eta[:], ltg[:, :ca], mybir.ActivationFunctionType.Exp,
                    scale=LN2,
                )
                etd = etdp.tile([CP, nd * BLK], f16, tag="etd")
                bits = bitp.tile([CP, nd * BLK], i32, tag="bits")
                nc.vector._custom_dve(
                    op1, out=bits[:], in0=ltg[:, ca:],
                    s0=KMAGIC, s1=P23, imm2=P23,
                )
                nc.vector._custom_dve(
                    op2, out=etd[:], in0=ltg[:, ca:],
                    in1=bits[:].bitcast(mybir.dt.float32),
                    s0=B1C, s1=B2C, imm2=KMAGIC,
                )
                for k in range(GSIZES[g]):
                    et = eta if k < na else etd
                    koff = k * BLK if k < na else (k - na) * BLK
                    sl, j = kk // SBLK, kk % SBLK
                    nc.tensor.matmul(
                        res[sl][:, j, 0:m],
                        et[:, koff : koff + BLK],
                        vs_v[:, m * kk : m * (kk + 1)],
                        start=True,
                        stop=True,
                    )
                    kk += 1
                while done < SLICES and kk >= (done + 1) * SBLK:
                    epilogue(done)
                    done += 1
            while done < SLICES:
                epilogue(done)
                done += 1

            # ---- SUM_{p,k} ln(denom) -> one f32 scalar
            lnr = fin.tile([BLK, NBLK], f32, tag="lnr")
            lsum = fin.tile([BLK, 1], f32, tag="lsum")
            nc.scalar.activation(
                lnr[:],
                lnsrc[:],
                mybir.ActivationFunctionType.Ln,
                accum_out=lsum[:],
            )
            tot_ps = resp.tile([1, 1], f32, tag="tot")
            nc.tensor.matmul(
                tot_ps[:], onesc[:], lsum[:], start=True, stop=True
            )
            tot_sb = fin.tile([1, 1], f32, tag="totsb")
            nc.vector.tensor_copy(tot_sb[:], tot_ps[:])
            nc.sync.dma_start(out_d[:], tot_sb[:])

    nc.compile()
    return nc


def _pick_stride(m: int) -> int:
    for st in (1, 2, 4, 8, 16):
        if st >= m and 512 % st == 0:
            return st
    raise ValueError(f"too many classes per block: m={m}")


def kernel(logits, s, targets):
    global LAST_RESULTS
    logits = np.asarray(logits, dtype=np.float32)
    s = np.asarray(s, dtype=np.float32)
    t = np.asarray(targets).astype(np.int64).ravel()
    assert logits.shape == (B, C) and s.shape == (C, C) and t.shape == (B,)

    order = np.argsort(t, kind="stable")
    # exact numerator sum on host: SUM_b logits[b, t_b]
    ztsum = float(logits[np.arange(B), t].sum(dtype=np.float64))

    idxs = [order[mm::NCORES] for mm in range(NCORES)]

    m = 1
    block_classes = []
    for idx in idxs:
        tb = t[idx].reshape(NBLK, BLK)
        cs = [np.unique(row) for row in tb]
        m = max(m, max(len(u) for u in cs))
        block_classes.append((tb, cs))
    stride = _pick_stride(m)

    # column ranges (in blocks) handled by the DVE engine per chunk
    bounds = np.cumsum([0] + GSIZES)
    dve_cols = []
    for g in range(NCHUNK):
        na = _act_blocks(GSIZES[g])
        dve_cols.append(((bounds[g] + na) * BLK, bounds[g + 1] * BLK))

    VSB = m * 256
    THB = (m - 1) * 256
    W0 = CCOLS + VSB + THB

    in_maps = []
    for core in range(NCORES):
        idx = idxs[core]
        tb, cs = block_classes[core]
        ltT = np.empty((CP, RPC), dtype=np.float32)
        ltT[:] = (logits[idx].T * LOG2E) + DCORR
        for a, b_ in dve_cols:
            ltT[:, a:b_] += LA
        lt8 = ltT.astype(ml_dtypes.float8_e4m3fn).view(np.uint8)  # [CP, RPC]

        vs = np.zeros((CP, m * NBLK), dtype=np.float16)
        cmat = np.empty((m, NBLK), dtype=np.int64)
        thr = np.full((m - 1, NBLK), BLK, dtype=np.float16)
        for k in range(NBLK):
            u = cs[k]
            cmat[: len(u), k] = u
            cmat[len(u):, k] = u[-1]
            # thresholds: first sample index of candidate i (sorted rows)
            pos = np.searchsorted(tb[k], u)
            for i in range(1, len(u)):
                thr[i - 1, k] = pos[i]
        for i in range(m):
            vs[:, i::m] = s[cmat[i]].T.astype(np.float16)

        p0 = np.zeros((CP, W0), dtype=np.uint8)
        p0[:, :CCOLS] = lt8[:, :CCOLS]
        p0[:, CCOLS : CCOLS + VSB] = vs.view(np.uint8)
        if m > 1:
            p0[0, CCOLS + VSB :] = thr.reshape(1, -1).view(np.uint8)
        im = {"p0": p0}
        for i in (1, 2, 3):
            im[f"c{i}"] = np.ascontiguousarray(
                lt8[:, bounds[i] * BLK : bounds[i + 1] * BLK]
            )
        in_maps.append(im)

    key = (m, stride)
    if key not in _nc_cache:
        _nc_cache[key] = _build_nc(m, stride)
    nc = _nc_cache[key]

    res = run_bass_kernel_spmd(
        nc, in_maps, core_ids=list(range(NCORES)), trace=TRACE
    )
    LAST_RESULTS = res
    lntot = sum(float(r["out"][0, 0]) for r in res.results)
    return np.float32((lntot - ztsum) / B - BIASCORR)
